# revision 38
# baseline (speedup 1.0000x reference)
"""Trainium2 bass kernel for nn_CM_41162966565199 (dense_cnn, dynamic filter).

Computation (per batch sample):
  filt = Conv2d(C=64 -> 9C=576, 3x3, pad=1)(gt) + bias          # dynamic filters
  out[c,h,w] = sum_j filt[c*9+j, h, w] * patches_j(gr)[c, h, w] # 3x3 dyn. filter

Strategy: pure data parallel, one sample per NeuronCore (N=8, 8 cores).

Final design (237us baseline -> ~183us):
- Matmul inputs bf16 (gt, weights): full PE rate, FWL weight loads, half the
  HBM bytes, and fewer multiplier mantissa toggles than fp16 (less exposure
  to the chip's data/power-dependent PE downclock, which costs a uniform
  ~20% when it strikes). Tolerance is 2e-2; end-to-end error is ~2e-3.
- Inputs live resident in SBUF as 4 pre-stacked [128, 17408] 16-bit buffers
  (gt/gr x delta-1/delta-132 partition-pair stacks), DMA'd ONCE in column
  chunks (18.6 MB total vs 42 MB streamed windows). After startup the PE
  never waits on DMA (~300 GB/s on big 128-partition chunk transfers).
- 33 spatial tiles of 512 (rows 1..128 of the padded grid), processed in
  blocks of 4 with a 4-deep PSUM m-stage pipeline.
- PE slot packing ~20.6 slots/tile vs the naive 25 (theoretical min 20.25):
  - K-chunk 4 (lone tap (2,2), K=64) runs as TWO CONCURRENT row-tiled K=64
    matmuls (tile_position (0,0)/(64,0)) covering two spatial tiles in one
    PE slot; weights duplicated into both partition halves, and the +1-
    shifted upper stack half supplies the second tile's window.
  - M-tile 4 (lone j-tap, M=64) runs as TWO CONCURRENT col-tiled M=64
    matmuls (tile_position (0,0)/(0,64)) covering two spatial tiles per
    slot, outputs on PSUM partition halves of the same columns; the host
    fold of the halves absorbs the layout. Its (c4, m4) corner uses
    tile_position (64,64). m4 runs FIRST so its short stage covers the
    previous block's PSUM drain.
- The (psum + bias) -> 16-bit stage runs on the otherwise-idle Scalar
  engine (activation Identity with per-partition bias AP, PSUM->SBUF).
  DVE keeps only the 5 gr-products + add tree per spatial pair (16-bit
  2x_1P ops). Out-DMA rides the ACT HWDGE queue; inputs the SP queue.
- A dummy-matmul train warms the PE HAM clock-gate (1.2->2.4 GHz) during
  the initial DMA wait, sized to hand off to the first data-ready matmuls.
- Spatial flattening uses a 2-ring padded 132x132 grid so every 3x3 tap is
  a pure flat offset; host pre-pads (zeros for conv, replicate ring for
  patches), stacks the shifted copies, and crops/folds the output.
"""

import ml_dtypes
import numpy as np

import concourse.bass as bass
import concourse.mybir as mybir
import concourse.tile as tile
from concourse import bacc
from concourse.bass_utils import run_bass_kernel_spmd
from concourse.vector_clock import ScopedClock

# ---------------------------------------------------------------- constants
N, C, H, W, KS = 8, 64, 128, 128, 3
W2 = W + 4                      # 132: 2-ring padded row width
NROW = H + 4                    # 132 padded rows
NTILE = 512
NT = 33                         # spatial tiles: rows 1..128 => 33*512 = 16896
OUT_LEN = NT * NTILE            # 16896
T0 = 132                        # first output flat position (row 1, col 0)
WSTK = 17408                    # stack width (covers max read 17294)

F32 = mybir.dt.float32
F16 = mybir.dt.float16
BF16 = mybir.dt.bfloat16
ADD = mybir.AluOpType.add
MULT = mybir.AluOpType.mult
IDENT = mybir.ActivationFunctionType.Identity

# 5 K-chunks over the 9 conv taps p=(kh,kw); flat offset d_p = kh*132+kw.
# Pairs (p_a, p_b): upper/lower SBUF partition halves. Chunks 0-2 pair
# (kh,0)+(kh,1) (delta=1, AB stack), chunk 3 pairs (0,2)+(1,2) (delta=132,
# AC stack), chunk 4 is the lone (2,2) with zeroed lower weights.
CHUNKS = [((0, 0), (0, 1)), ((1, 0), (1, 1)), ((2, 0), (2, 1)),
          ((0, 2), (1, 2)), ((2, 2), None)]
# 5 M-tiles: which two j-groups (of the 9 output filter taps) share a PSUM
# tile's upper/lower 64 partitions.
MTILES = CHUNKS


# ------------------------------------------------- TileContext drain patch
# This walrus build rejects >2 sync-wait commands on one CTRL instruction;
# the stock TileContext tail hangs every pending sem wait on a single SP
# Drain. Split them across single-wait SP NOPs (program order on SP still
# places them before the barrier + sem reset).
def _drain_and_barrier_split(self, tick_clock, wait_clock):
    nc = self.nc
    drain_inst = nc.sync.drain()
    wait_clock.add_sem_waits(
        drain_inst.ins, ScopedClock({None: tick_clock.global_clock})
    )
    si = drain_inst.ins.sync_info
    if si is not None and len(si.on_wait) > 1:
        waits = list(si.on_wait)
        drain_inst.ins.sync_info = mybir.SyncInfo(on_wait=[waits[0]], on_update=[])
        for w in waits[1:]:
            nop = nc.sync.nop()
            nop.ins.sync_info = mybir.SyncInfo(on_wait=[w], on_update=[])
    nc.all_engine_barrier()
    assert self.sems is not None
    popped = nc._tile_sem_poison_stack.pop()
    assert popped is self._sem_poison
    nc.clear_and_free_semaphores(list(self.sems.allocated().values()))
    nc.all_engine_barrier()


tile.TileContext._drain_and_barrier = _drain_and_barrier_split


# ------------------------------------------------------------- host prep
def _flat_gt(gt):
    """[C,H,W] -> [C, WSTK+136] flat 132x132 grid, 2-ring zero pad."""
    pad = np.zeros((C, NROW, W2), np.float32)
    pad[:, 2:2 + H, 2:2 + W] = gt
    buf = np.zeros((C, WSTK + 136), np.float32)
    buf[:, :NROW * W2] = pad.reshape(C, -1)
    return buf


def _flat_gr(gr):
    """[C,H,W] -> flat 132x132 grid; inner 130x130 = replicate-padded gr."""
    rp = np.pad(gr, ((0, 0), (1, 1), (1, 1)), mode="edge")
    pad = np.zeros((C, NROW, W2), np.float32)
    pad[:, 1:3 + H, 1:3 + W] = rp
    buf = np.zeros((C, WSTK + 136), np.float32)
    buf[:, :NROW * W2] = pad.reshape(C, -1)
    return buf


def _stack(flat, d0, d1, dtype=np.float16):
    """[C, >=WSTK+d1] -> [128, WSTK]: partitions 0-63 <- flat[:, d0+y],
    64-127 <- flat[:, d1+y]."""
    out = np.empty((2 * C, WSTK), dtype)
    out[:C] = flat[:, d0:d0 + WSTK]
    out[C:] = flat[:, d1:d1 + WSTK]
    return out


def _jidx(j):
    return j[0] * 3 + j[1]


def _prep_w(Wc):
    """[576,64,3,3] -> [128, 25*128] fp16 lhsT blocks [(m,c), K, M],
    partition-major."""
    out = np.zeros((5, 5, 128, 128), np.float32)
    cc = np.arange(C)
    for m, (j0, j1) in enumerate(MTILES):
        for c, (pa, pb) in enumerate(CHUNKS):
            # chunk 4 (lone tap): duplicate the K=64 weights into both
            # partition halves — the kernel runs it as two concurrent
            # row-tiled K=64 matmuls covering two spatial tiles
            taps = ((0, pa), (1, pb)) if pb is not None else ((0, pa), (1, pa))
            for hk, p in taps:
                kh, kw = p
                for hm, j in ((0, j0), (1, j1)):
                    if j is None:
                        continue
                    blk = Wc[cc * 9 + _jidx(j), :, kh, kw]  # [c_out, i]
                    out[m, c, 64 * hk:64 * hk + 64, 64 * hm:64 * hm + 64] = blk.T
    # M-tile 4 (lone j-tap, M=64): duplicate into columns 64-127 — the kernel
    # runs m4 as two concurrent col-tiled M=64 matmuls covering two spatial
    # tiles (outputs on partition halves; the host fold adds both halves)
    out[4, :, :, 64:128] = out[4, :, :, 0:64]
    return np.ascontiguousarray(
        out.reshape(25, 128, 128).transpose(1, 0, 2).reshape(128, 25 * 128)
    ).astype(ml_dtypes.bfloat16)


def _prep_b(bc):
    """[576] -> [128,5] per-M-tile per-partition bias (partition-major)."""
    out = np.zeros((5, 128), np.float32)
    cc = np.arange(C)
    for m, (j0, j1) in enumerate(MTILES):
        for hm, j in ((0, j0), (1, j1)):
            if j is None:
                continue
            out[m, 64 * hm:64 * hm + 64] = bc[cc * 9 + _jidx(j)]
    out[4, 64:128] = out[4, 0:64]  # m4 bias on both partition halves
    return np.ascontiguousarray(out.T)


# --------------------------------------------------------- bass program
# stack column-chunk boundaries: small first chunks for a fast PE start
CHUNK_EDGES = [0, 2048, 4096, 6144, 8192, 10240, 12288, 14336, 16384, WSTK]
# spatial-tile block sizes; m-stages run in order [m4, m0..m3] so the short
# m4 stage covers the previous block's last PSUM drain (Scalar-engine ACT)
BLOCKS = [1, 2, 4, 4, 4, 4, 4, 4, 4, 2]
assert sum(BLOCKS) == NT


def _build():
    # Bacc (not plain Bass): its finalize() -> compile() legalizes the
    # multi-wait instructions Tile emits (move_matmul_waits_to_ldweights,
    # generate_event_semaphores) which this walrus build otherwise rejects
    # with "Too many sync wait commands".
    nc = bacc.Bacc(None, target_bir_lowering=False)
    gtab_src = nc.dram_tensor("gtab_src", [128, WSTK], BF16, kind="ExternalInput")
    gtac_src = nc.dram_tensor("gtac_src", [128, WSTK], BF16, kind="ExternalInput")
    grab_src = nc.dram_tensor("grab_src", [128, WSTK], F16, kind="ExternalInput")
    grac_src = nc.dram_tensor("grac_src", [128, WSTK], F16, kind="ExternalInput")
    w_src = nc.dram_tensor("w_src", [128, 25 * 128], BF16, kind="ExternalInput")
    b_src = nc.dram_tensor("b_src", [128, 5], F32, kind="ExternalInput")
    o_dst = nc.dram_tensor("o_dst", [128, OUT_LEN], F16, kind="ExternalOutput")

    with tile.TileContext(nc) as tc:
        with (
            tc.tile_pool(name="stkpool", bufs=1) as stkpool,
            tc.tile_pool(name="wpool", bufs=1) as wpool,
            tc.tile_pool(name="dumpool", bufs=1) as dumpool,
            tc.tile_pool(name="pspool", bufs=4, space="PSUM") as pspool,
            tc.tile_pool(name="fpool", bufs=8) as fpool,
            tc.tile_pool(name="prodpool", bufs=12) as prodpool,
            tc.tile_pool(name="accpool", bufs=6) as accpool,
        ):
            gtab = stkpool.tile([128, WSTK], BF16, name="gtab", tag="gtab")
            gtac = stkpool.tile([128, WSTK], BF16, name="gtac", tag="gtac")
            grab = stkpool.tile([128, WSTK], F16, name="grab", tag="grab")
            grac = stkpool.tile([128, WSTK], F16, name="grac", tag="grac")
            wsb = wpool.tile([128, 25 * 128], BF16, name="wsb", tag="wsb")
            bias_sb = wpool.tile([128, 5], F32, name="bias_sb", tag="bias")

            # PE warmup: dummy matmuls (zero x zero) to flip the HAM clock
            # gate to 2.4 GHz while the first input chunks are in flight.
            dummy = dumpool.tile([128, 640], BF16, name="dummy", tag="dummy")
            nc.vector.memset(dummy[:, :], 0.0)
            warm_ps = pspool.tile([128, 1024], F32, name="warm_ps", tag="ps")
            for i in range(15):
                nc.tensor.matmul(
                    warm_ps[:, 0:512], dummy[:, 0:128], dummy[:, 128:640],
                    start=True, stop=True,
                )

            # input DMAs on the SP queue, ordered for the earliest PE start:
            # m4 weights (the m4-first stage runs first), first gt chunks,
            # remaining weights, then round-robin chunks of the four stacks
            nc.sync.dma_start(out=wsb[:, 2560:3200], in_=w_src[:, 2560:3200])
            lo, hi = CHUNK_EDGES[0], CHUNK_EDGES[1]
            nc.sync.dma_start(out=gtab[:, lo:hi], in_=gtab_src[:, lo:hi])
            nc.sync.dma_start(out=gtac[:, lo:hi], in_=gtac_src[:, lo:hi])
            nc.sync.dma_start(out=wsb[:, 0:2560], in_=w_src[:, 0:2560])
            nc.sync.dma_start(out=bias_sb[:, :], in_=b_src[:, :])
            nc.sync.dma_start(out=grab[:, lo:hi], in_=grab_src[:, lo:hi])
            nc.sync.dma_start(out=grac[:, lo:hi], in_=grac_src[:, lo:hi])
            for ci in range(1, len(CHUNK_EDGES) - 1):
                lo, hi = CHUNK_EDGES[ci], CHUNK_EDGES[ci + 1]
                for t, src in ((gtab, gtab_src), (gtac, gtac_src),
                               (grab, grab_src), (grac, grac_src)):
                    nc.sync.dma_start(out=t[:, lo:hi], in_=src[:, lo:hi])

            def emit_mult(m, T, Wd, f):
                """prod = f * gr-patch for M-tile m (0-3) of the pair at T."""
                pr = prodpool.tile([128, 1024], F16, name=f"pr{m}",
                                   tag="prod")
                if m < 3:
                    g = grab[:, T + m * W2: T + m * W2 + Wd]
                else:
                    g = grac[:, T: T + Wd]
                nc.vector.tensor_tensor(pr[:, 0:Wd], f[:, 0:Wd], g, op=MULT)
                return pr

            def emit_tree(T, Wd, prods):
                """fp16 pairwise sum tree on DVE + out-DMA for one pair.
                m4's product sits on partitions 0-63 for the pair's first
                tile and 64-127 for the second (col-tiled quadrant layout);
                the host fold of the two partition halves absorbs this."""
                p0, p1, p2, p3, p4 = prods
                a1 = accpool.tile([128, 1024], F16, name="a1", tag="acc")
                nc.vector.tensor_tensor(a1[:, 0:Wd], p0[:, 0:Wd], p1[:, 0:Wd],
                                        op=ADD)
                a2 = accpool.tile([128, 1024], F16, name="a2", tag="acc")
                nc.vector.tensor_tensor(a2[:, 0:Wd], p2[:, 0:Wd], p3[:, 0:Wd],
                                        op=ADD)
                a3 = accpool.tile([128, 1024], F16, name="a3", tag="acc")
                nc.vector.tensor_tensor(a3[:, 0:Wd], a1[:, 0:Wd], a2[:, 0:Wd],
                                        op=ADD)
                nc.vector.tensor_tensor(a3[0:64, 0:NTILE], a3[0:64, 0:NTILE],
                                        p4[0:64, 0:NTILE], op=ADD)
                if Wd == 2 * NTILE:
                    nc.vector.tensor_tensor(
                        a3[64:128, NTILE:2 * NTILE],
                        a3[64:128, NTILE:2 * NTILE],
                        p4[64:128, 0:NTILE], op=ADD)
                # out-DMA from the ACT queue (SP queue is busy with inputs)
                nc.scalar.dma_start(
                    out=o_dst[:, T - T0: T - T0 + Wd], in_=a3[:, 0:Wd]
                )

            # blocks of nb spatial tiles: per (m, c) the nb matmuls share one
            # stationary-weight load (the PE only pays the weight-swap drain
            # once per nb); downstream runs per pair of tiles (1024 cols)
            t0 = 0
            for nb in BLOCKS:
                T = T0 + t0 * NTILE
                npair = (nb + 1) // 2
                wds = [min(2 * NTILE, (nb - 2 * p) * NTILE)
                       for p in range(npair)]
                prodss = [[None] * 5 for _ in range(npair)]

                def rhs_c(c, q):
                    if c < 3:
                        return gtab[:, q + c * W2: q + c * W2 + NTILE]
                    return gtac[:, q: q + NTILE]

                # m4 (lone j-tap, M=64) first: two concurrent col-tiled M=64
                # matmuls per (c, tile-pair) sharing ONE [128,512] PSUM region
                # (the pair's first tile on partitions 0-63, the second on
                # 64-127, same columns; weight cols 64-127 hold the duplicate)
                pst = [pspool.tile([128, 1024], F32, name=f"ps4_{p}",
                                   tag="ps") for p in range(npair)]
                for c in range(5):
                    k = 4 * 5 + c
                    wlo = wsb[:, k * 128: k * 128 + 64]
                    whi = wsb[:, k * 128 + 64: (k + 1) * 128]
                    tb = 0
                    while tb < nb:
                        q = T + tb * NTILE
                        pt = pst[tb // 2]
                        if tb + 1 < nb:
                            q1 = q + NTILE
                            if c < 4:
                                nc.tensor.matmul(
                                    pt[0:64, 0:NTILE], wlo, rhs_c(c, q),
                                    start=(c == 0), stop=False,
                                    tile_position=(0, 0),
                                )
                                nc.tensor.matmul(
                                    pt[64:128, 0:NTILE], whi, rhs_c(c, q1),
                                    start=(c == 0), stop=False,
                                    tile_position=(0, 64),
                                )
                            else:
                                nc.tensor.matmul(
                                    pt[0:64, 0:NTILE], wlo[0:64, :],
                                    gtab[0:64, q + 266: q + 266 + NTILE],
                                    start=False, stop=True,
                                    tile_position=(0, 0),
                                )
                                nc.tensor.matmul(
                                    pt[64:128, 0:NTILE], whi[64:128, :],
                                    gtab[64:128, q1 + 265: q1 + 265 + NTILE],
                                    start=False, stop=True,
                                    tile_position=(64, 64),
                                )
                            tb += 2
                        else:
                            # lone tile: old-style full-width, lower half used
                            if c < 4:
                                nc.tensor.matmul(
                                    pt[:, 0:NTILE],
                                    wsb[:, k * 128:(k + 1) * 128],
                                    rhs_c(c, q), start=(c == 0), stop=False,
                                )
                            else:
                                nc.tensor.matmul(
                                    pt[:, 0:NTILE],
                                    wsb[0:64, k * 128:(k + 1) * 128],
                                    gtab[0:64, q + 266: q + 266 + NTILE],
                                    start=False, stop=True,
                                )
                            tb += 1
                for p in range(npair):
                    Tp = T + p * 2 * NTILE
                    f = fpool.tile([128, 1024], F16, name=f"f4_{p}", tag="f")
                    pr = prodpool.tile([128, 1024], F16, name="pr4",
                                       tag="prod")
                    nc.scalar.activation(
                        f[:, 0:NTILE], pst[p][:, 0:NTILE], IDENT,
                        bias=bias_sb[:, 4:5],
                    )
                    nc.vector.tensor_tensor(
                        pr[0:64, 0:NTILE], f[0:64, 0:NTILE],
                        grab[0:64, Tp + 266: Tp + 266 + NTILE], op=MULT)
                    if wds[p] == 2 * NTILE:
                        nc.vector.tensor_tensor(
                            pr[64:128, 0:NTILE], f[64:128, 0:NTILE],
                            grab[64:128, Tp + NTILE + 265:
                                 Tp + NTILE + 265 + NTILE], op=MULT)
                    prodss[p][4] = pr

                for m in range(4):
                    pst = [pspool.tile([128, 1024], F32, name=f"ps{m}_{p}",
                                       tag="ps") for p in range(npair)]
                    for c in range(5):
                        k = m * 5 + c
                        lhsT = wsb[:, k * 128:(k + 1) * 128]
                        if c < 4:
                            for tb in range(nb):
                                out_ps = pst[tb // 2][:, (tb % 2) * NTILE:
                                                      (tb % 2 + 1) * NTILE]
                                nc.tensor.matmul(
                                    out_ps, lhsT, rhs_c(c, T + tb * NTILE),
                                    start=(c == 0), stop=False,
                                )
                        else:
                            # lone tap (2,2): two concurrent row-tiled K=64
                            # matmuls (partitions 0-63 / 64-127 hold the same
                            # weights; upper stack half is gt shifted +1, so
                            # column q+265 reads gt[q+266]) — two spatial
                            # tiles in one PE slot
                            tb = 0
                            while tb < nb:
                                q = T + tb * NTILE
                                out_ps = pst[tb // 2][:, (tb % 2) * NTILE:
                                                      (tb % 2 + 1) * NTILE]
                                if tb + 1 < nb:
                                    q1 = T + (tb + 1) * NTILE
                                    out_b = pst[(tb + 1) // 2][
                                        :, ((tb + 1) % 2) * NTILE:
                                        ((tb + 1) % 2 + 1) * NTILE]
                                    nc.tensor.matmul(
                                        out_ps, lhsT[0:64, :],
                                        gtab[0:64, q + 266: q + 266 + NTILE],
                                        start=False, stop=True,
                                        tile_position=(0, 0),
                                    )
                                    nc.tensor.matmul(
                                        out_b, lhsT[64:128, :],
                                        gtab[64:128, q1 + 265:
                                             q1 + 265 + NTILE],
                                        start=False, stop=True,
                                        tile_position=(64, 0),
                                    )
                                    tb += 2
                                else:
                                    nc.tensor.matmul(
                                        out_ps, lhsT[0:64, :],
                                        gtab[0:64, q + 266: q + 266 + NTILE],
                                        start=False, stop=True,
                                    )
                                    tb += 1
                    # psum + bias -> fp16 SBUF on the Scalar engine, then the
                    # gr-product immediately on DVE. The last stage (m3)
                    # instead fuses (psum + bias) * gr into one DVE
                    # scalar_tensor_tensor: it shortens the ACT queue by
                    # ~2.2us per block so the block-boundary PSUM reuse
                    # never waits on the Scalar engine.
                    for p in range(npair):
                        Wd = wds[p]
                        Tp = T + p * 2 * NTILE
                        if m == 3:
                            pr = prodpool.tile([128, 1024], F16, name="pr3",
                                               tag="prod")
                            nc.vector.scalar_tensor_tensor(
                                pr[:, 0:Wd], pst[p][:, 0:Wd],
                                bias_sb[:, 3:4], grac[:, Tp: Tp + Wd],
                                op0=ADD, op1=MULT,
                            )
                            prodss[p][m] = pr
                        else:
                            f = fpool.tile([128, 1024], F16, name=f"f{m}_{p}",
                                           tag="f")
                            nc.scalar.activation(
                                f[:, 0:Wd], pst[p][:, 0:Wd], IDENT,
                                bias=bias_sb[:, m:m + 1],
                            )
                            prodss[p][m] = emit_mult(m, Tp, Wd, f)

                for p in range(npair):
                    emit_tree(T + p * 2 * NTILE, wds[p], prodss[p])
                t0 += nb
    nc.finalize()
    return nc


_NC = None


def _get_nc():
    global _NC
    if _NC is None:
        _NC = _build()
    return _NC


_RUN_KW = {}  # test harness can inject trace=True etc.
_LAST_RESULT = None


def kernel(gr, gt, Wc, bc):
    global _LAST_RESULT
    gr = np.ascontiguousarray(np.asarray(gr, dtype=np.float32))
    gt = np.ascontiguousarray(np.asarray(gt, dtype=np.float32))
    Wc = np.asarray(Wc, dtype=np.float32)
    bc = np.asarray(bc, dtype=np.float32)

    wb = _prep_w(Wc)
    bb = _prep_b(bc)
    in_maps = []
    for n in range(N):
        fgt = _flat_gt(gt[n])
        fgr = _flat_gr(gr[n])
        in_maps.append({
            "gtab_src": _stack(fgt, 0, 1, ml_dtypes.bfloat16),
            "gtac_src": _stack(fgt, 2, 134, ml_dtypes.bfloat16),
            "grab_src": _stack(fgr, 0, 1),
            "grac_src": _stack(fgr, 2, 134),
            "w_src": wb,
            "b_src": bb,
        })
    res = run_bass_kernel_spmd(
        _get_nc(), in_maps, core_ids=list(range(N)), **_RUN_KW
    )
    _LAST_RESULT = res

    # o_dst col t <-> flat pos 132 + t; output (h,w) at flat (h+1)*132+(w+1)
    hh = np.arange(H)
    cols = (hh * W2)[:, None] + (np.arange(W) + 1)[None, :]
    outs = []
    for n in range(N):
        O = res.results[n]["o_dst"].astype(np.float32)
        flat = O[:64] + O[64:]
        outs.append(flat[:, cols])
    return np.stack(outs).astype(np.float32)


# revision 39
# speedup vs baseline: 1.0109x; 1.0109x over previous
"""Trainium2 bass kernel for nn_CM_41162966565199 (dense_cnn, dynamic filter).

Computation (per batch sample):
  filt = Conv2d(C=64 -> 9C=576, 3x3, pad=1)(gt) + bias          # dynamic filters
  out[c,h,w] = sum_j filt[c*9+j, h, w] * patches_j(gr)[c, h, w] # 3x3 dyn. filter

Strategy: pure data parallel, one sample per NeuronCore (N=8, 8 cores).

Final design (237us baseline -> ~183us):
- Matmul inputs bf16 (gt, weights): full PE rate, FWL weight loads, half the
  HBM bytes, and fewer multiplier mantissa toggles than fp16 (less exposure
  to the chip's data/power-dependent PE downclock, which costs a uniform
  ~20% when it strikes). Tolerance is 2e-2; end-to-end error is ~2e-3.
- Inputs live resident in SBUF as 4 pre-stacked [128, 17408] 16-bit buffers
  (gt/gr x delta-1/delta-132 partition-pair stacks), DMA'd ONCE in column
  chunks (18.6 MB total vs 42 MB streamed windows). After startup the PE
  never waits on DMA (~300 GB/s on big 128-partition chunk transfers).
- 33 spatial tiles of 512 (rows 1..128 of the padded grid), processed in
  blocks of 4 with a 4-deep PSUM m-stage pipeline.
- PE slot packing ~20.6 slots/tile vs the naive 25 (theoretical min 20.25):
  - K-chunk 4 (lone tap (2,2), K=64) runs as TWO CONCURRENT row-tiled K=64
    matmuls (tile_position (0,0)/(64,0)) covering two spatial tiles in one
    PE slot; weights duplicated into both partition halves, and the +1-
    shifted upper stack half supplies the second tile's window.
  - M-tile 4 (lone j-tap, M=64) runs as TWO CONCURRENT col-tiled M=64
    matmuls (tile_position (0,0)/(0,64)) covering two spatial tiles per
    slot, outputs on PSUM partition halves of the same columns; the host
    fold of the halves absorbs the layout. Its (c4, m4) corner uses
    tile_position (64,64). m4 runs FIRST so its short stage covers the
    previous block's PSUM drain.
- The (psum + bias) -> 16-bit stage runs on the otherwise-idle Scalar
  engine (activation Identity with per-partition bias AP, PSUM->SBUF).
  DVE keeps only the 5 gr-products + add tree per spatial pair (16-bit
  2x_1P ops). Out-DMA rides the ACT HWDGE queue; inputs the SP queue.
- A dummy-matmul train warms the PE HAM clock-gate (1.2->2.4 GHz) during
  the initial DMA wait, sized to hand off to the first data-ready matmuls.
- Spatial flattening uses a 2-ring padded 132x132 grid so every 3x3 tap is
  a pure flat offset; host pre-pads (zeros for conv, replicate ring for
  patches), stacks the shifted copies, and crops/folds the output.
"""

import ml_dtypes
import numpy as np

import concourse.bass as bass
import concourse.mybir as mybir
import concourse.tile as tile
from concourse import bacc
from concourse.bass_utils import run_bass_kernel_spmd
from concourse.vector_clock import ScopedClock

# ---------------------------------------------------------------- constants
N, C, H, W, KS = 8, 64, 128, 128, 3
W2 = W + 4                      # 132: 2-ring padded row width
NROW = H + 4                    # 132 padded rows
NTILE = 512
NT = 33                         # spatial tiles: rows 1..128 => 33*512 = 16896
OUT_LEN = NT * NTILE            # 16896
T0 = 132                        # first output flat position (row 1, col 0)
WSTK = 17408                    # stack width (covers max read 17294)

F32 = mybir.dt.float32
F16 = mybir.dt.float16
BF16 = mybir.dt.bfloat16
ADD = mybir.AluOpType.add
MULT = mybir.AluOpType.mult
IDENT = mybir.ActivationFunctionType.Identity

# 5 K-chunks over the 9 conv taps p=(kh,kw); flat offset d_p = kh*132+kw.
# Pairs (p_a, p_b): upper/lower SBUF partition halves. Chunks 0-2 pair
# (kh,0)+(kh,1) (delta=1, AB stack), chunk 3 pairs (0,2)+(1,2) (delta=132,
# AC stack), chunk 4 is the lone (2,2) with zeroed lower weights.
CHUNKS = [((0, 0), (0, 1)), ((1, 0), (1, 1)), ((2, 0), (2, 1)),
          ((0, 2), (1, 2)), ((2, 2), None)]
# 5 M-tiles: which two j-groups (of the 9 output filter taps) share a PSUM
# tile's upper/lower 64 partitions.
MTILES = CHUNKS


# ------------------------------------------------- TileContext drain patch
# This walrus build rejects >2 sync-wait commands on one CTRL instruction;
# the stock TileContext tail hangs every pending sem wait on a single SP
# Drain. Split them across single-wait SP NOPs (program order on SP still
# places them before the barrier + sem reset).
def _drain_and_barrier_split(self, tick_clock, wait_clock):
    nc = self.nc
    drain_inst = nc.sync.drain()
    wait_clock.add_sem_waits(
        drain_inst.ins, ScopedClock({None: tick_clock.global_clock})
    )
    si = drain_inst.ins.sync_info
    if si is not None and len(si.on_wait) > 1:
        waits = list(si.on_wait)
        drain_inst.ins.sync_info = mybir.SyncInfo(on_wait=[waits[0]], on_update=[])
        for w in waits[1:]:
            nop = nc.sync.nop()
            nop.ins.sync_info = mybir.SyncInfo(on_wait=[w], on_update=[])
    nc.all_engine_barrier()
    assert self.sems is not None
    popped = nc._tile_sem_poison_stack.pop()
    assert popped is self._sem_poison
    nc.clear_and_free_semaphores(list(self.sems.allocated().values()))
    nc.all_engine_barrier()


tile.TileContext._drain_and_barrier = _drain_and_barrier_split


# ------------------------------------------------------------- host prep
def _flat_gt(gt):
    """[C,H,W] -> [C, WSTK+136] flat 132x132 grid, 2-ring zero pad."""
    pad = np.zeros((C, NROW, W2), np.float32)
    pad[:, 2:2 + H, 2:2 + W] = gt
    buf = np.zeros((C, WSTK + 136), np.float32)
    buf[:, :NROW * W2] = pad.reshape(C, -1)
    return buf


def _flat_gr(gr):
    """[C,H,W] -> flat 132x132 grid; inner 130x130 = replicate-padded gr."""
    rp = np.pad(gr, ((0, 0), (1, 1), (1, 1)), mode="edge")
    pad = np.zeros((C, NROW, W2), np.float32)
    pad[:, 1:3 + H, 1:3 + W] = rp
    buf = np.zeros((C, WSTK + 136), np.float32)
    buf[:, :NROW * W2] = pad.reshape(C, -1)
    return buf


def _stack(flat, d0, d1, dtype=np.float16):
    """[C, >=WSTK+d1] -> [128, WSTK]: partitions 0-63 <- flat[:, d0+y],
    64-127 <- flat[:, d1+y]."""
    out = np.empty((2 * C, WSTK), dtype)
    out[:C] = flat[:, d0:d0 + WSTK]
    out[C:] = flat[:, d1:d1 + WSTK]
    return out


def _jidx(j):
    return j[0] * 3 + j[1]


def _prep_w(Wc):
    """[576,64,3,3] -> [128, 25*128] fp16 lhsT blocks [(m,c), K, M],
    partition-major."""
    out = np.zeros((5, 5, 128, 128), np.float32)
    cc = np.arange(C)
    for m, (j0, j1) in enumerate(MTILES):
        for c, (pa, pb) in enumerate(CHUNKS):
            # chunk 4 (lone tap): duplicate the K=64 weights into both
            # partition halves — the kernel runs it as two concurrent
            # row-tiled K=64 matmuls covering two spatial tiles
            taps = ((0, pa), (1, pb)) if pb is not None else ((0, pa), (1, pa))
            for hk, p in taps:
                kh, kw = p
                for hm, j in ((0, j0), (1, j1)):
                    if j is None:
                        continue
                    blk = Wc[cc * 9 + _jidx(j), :, kh, kw]  # [c_out, i]
                    out[m, c, 64 * hk:64 * hk + 64, 64 * hm:64 * hm + 64] = blk.T
    # M-tile 4 (lone j-tap, M=64): duplicate into columns 64-127 — the kernel
    # runs m4 as two concurrent col-tiled M=64 matmuls covering two spatial
    # tiles (outputs on partition halves; the host fold adds both halves)
    out[4, :, :, 64:128] = out[4, :, :, 0:64]
    return np.ascontiguousarray(
        out.reshape(25, 128, 128).transpose(1, 0, 2).reshape(128, 25 * 128)
    ).astype(ml_dtypes.bfloat16)


def _prep_b(bc):
    """[576] -> [128,5] per-M-tile per-partition bias (partition-major)."""
    out = np.zeros((5, 128), np.float32)
    cc = np.arange(C)
    for m, (j0, j1) in enumerate(MTILES):
        for hm, j in ((0, j0), (1, j1)):
            if j is None:
                continue
            out[m, 64 * hm:64 * hm + 64] = bc[cc * 9 + _jidx(j)]
    out[4, 64:128] = out[4, 0:64]  # m4 bias on both partition halves
    return np.ascontiguousarray(out.T)


# --------------------------------------------------------- bass program
# stack column-chunk boundaries: small first chunks for a fast PE start
CHUNK_EDGES = [0, 2048, 4096, 6144, 8192, 10240, 12288, 14336, 16384, WSTK]
# spatial-tile block sizes; m-stages run in order [m4, m0..m3] so the short
# m4 stage covers the previous block's last PSUM drain (Scalar-engine ACT)
BLOCKS = [1, 2, 4, 4, 4, 4, 4, 4, 4, 2]
assert sum(BLOCKS) == NT


def _build():
    # Bacc (not plain Bass): its finalize() -> compile() legalizes the
    # multi-wait instructions Tile emits (move_matmul_waits_to_ldweights,
    # generate_event_semaphores) which this walrus build otherwise rejects
    # with "Too many sync wait commands".
    nc = bacc.Bacc(None, target_bir_lowering=False)
    gtab_src = nc.dram_tensor("gtab_src", [128, WSTK], BF16, kind="ExternalInput")
    gtac_src = nc.dram_tensor("gtac_src", [128, WSTK], BF16, kind="ExternalInput")
    grab_src = nc.dram_tensor("grab_src", [128, WSTK], F16, kind="ExternalInput")
    grac_src = nc.dram_tensor("grac_src", [128, WSTK], F16, kind="ExternalInput")
    w_src = nc.dram_tensor("w_src", [128, 25 * 128], BF16, kind="ExternalInput")
    b_src = nc.dram_tensor("b_src", [128, 5], F32, kind="ExternalInput")
    o_dst = nc.dram_tensor("o_dst", [128, OUT_LEN], F16, kind="ExternalOutput")

    with tile.TileContext(nc) as tc:
        with (
            tc.tile_pool(name="stkpool", bufs=1) as stkpool,
            tc.tile_pool(name="wpool", bufs=1) as wpool,
            tc.tile_pool(name="dumpool", bufs=1) as dumpool,
            tc.tile_pool(name="pspool", bufs=4, space="PSUM") as pspool,
            tc.tile_pool(name="fpool", bufs=8) as fpool,
            tc.tile_pool(name="prodpool", bufs=12) as prodpool,
            tc.tile_pool(name="accpool", bufs=6) as accpool,
        ):
            gtab = stkpool.tile([128, WSTK], BF16, name="gtab", tag="gtab")
            gtac = stkpool.tile([128, WSTK], BF16, name="gtac", tag="gtac")
            grab = stkpool.tile([128, WSTK], F16, name="grab", tag="grab")
            grac = stkpool.tile([128, WSTK], F16, name="grac", tag="grac")
            wsb = wpool.tile([128, 25 * 128], BF16, name="wsb", tag="wsb")
            bias_sb = wpool.tile([128, 5], F32, name="bias_sb", tag="bias")

            # PE warmup: dummy matmuls (zero x zero) to flip the HAM clock
            # gate to 2.4 GHz while the first input chunks are in flight.
            dummy = dumpool.tile([128, 640], BF16, name="dummy", tag="dummy")
            nc.vector.memset(dummy[:, :], 0.0)
            warm_ps = pspool.tile([128, 1024], F32, name="warm_ps", tag="ps")
            for i in range(15):
                nc.tensor.matmul(
                    warm_ps[:, 0:512], dummy[:, 0:128], dummy[:, 128:640],
                    start=True, stop=True,
                )

            # input DMAs on the SP queue, ordered for the earliest PE start:
            # m4 weights (the m4-first stage runs first), first gt chunks,
            # remaining weights, then round-robin chunks of the four stacks
            nc.sync.dma_start(out=wsb[:, 2560:3200], in_=w_src[:, 2560:3200])
            lo, hi = CHUNK_EDGES[0], CHUNK_EDGES[1]
            nc.sync.dma_start(out=gtab[:, lo:hi], in_=gtab_src[:, lo:hi])
            nc.sync.dma_start(out=gtac[:, lo:hi], in_=gtac_src[:, lo:hi])
            nc.sync.dma_start(out=wsb[:, 0:2560], in_=w_src[:, 0:2560])
            nc.sync.dma_start(out=bias_sb[:, :], in_=b_src[:, :])
            nc.sync.dma_start(out=grab[:, lo:hi], in_=grab_src[:, lo:hi])
            nc.sync.dma_start(out=grac[:, lo:hi], in_=grac_src[:, lo:hi])
            for ci in range(1, len(CHUNK_EDGES) - 1):
                lo, hi = CHUNK_EDGES[ci], CHUNK_EDGES[ci + 1]
                for t, src in ((gtab, gtab_src), (gtac, gtac_src),
                               (grab, grab_src), (grac, grac_src)):
                    nc.sync.dma_start(out=t[:, lo:hi], in_=src[:, lo:hi])

            def emit_mult(m, T, Wd, f):
                """prod = f * gr-patch for M-tile m (0-3) of the pair at T."""
                pr = prodpool.tile([128, 1024], F16, name=f"pr{m}",
                                   tag="prod")
                if m < 3:
                    g = grab[:, T + m * W2: T + m * W2 + Wd]
                else:
                    g = grac[:, T: T + Wd]
                nc.vector.tensor_tensor(pr[:, 0:Wd], f[:, 0:Wd], g, op=MULT)
                return pr

            def emit_tree(T, Wd, prods):
                """fp16 pairwise sum tree on DVE + out-DMA for one pair.
                m4's product sits on partitions 0-63 for the pair's first
                tile and 64-127 for the second (col-tiled quadrant layout);
                the host fold of the two partition halves absorbs this."""
                p0, p1, p2, p3, p4 = prods
                a1 = accpool.tile([128, 1024], F16, name="a1", tag="acc")
                nc.vector.tensor_tensor(a1[:, 0:Wd], p0[:, 0:Wd], p1[:, 0:Wd],
                                        op=ADD)
                a2 = accpool.tile([128, 1024], F16, name="a2", tag="acc")
                nc.vector.tensor_tensor(a2[:, 0:Wd], p2[:, 0:Wd], p3[:, 0:Wd],
                                        op=ADD)
                a3 = accpool.tile([128, 1024], F16, name="a3", tag="acc")
                nc.vector.tensor_tensor(a3[:, 0:Wd], a1[:, 0:Wd], a2[:, 0:Wd],
                                        op=ADD)
                nc.vector.tensor_tensor(a3[0:64, 0:NTILE], a3[0:64, 0:NTILE],
                                        p4[0:64, 0:NTILE], op=ADD)
                if Wd == 2 * NTILE:
                    nc.vector.tensor_tensor(
                        a3[64:128, NTILE:2 * NTILE],
                        a3[64:128, NTILE:2 * NTILE],
                        p4[64:128, 0:NTILE], op=ADD)
                # out-DMA from the ACT queue (SP queue is busy with inputs)
                nc.scalar.dma_start(
                    out=o_dst[:, T - T0: T - T0 + Wd], in_=a3[:, 0:Wd]
                )

            # blocks of nb spatial tiles: per (m, c) the nb matmuls share one
            # stationary-weight load (the PE only pays the weight-swap drain
            # once per nb); downstream runs per pair of tiles (1024 cols)
            t0 = 0
            for nb in BLOCKS:
                T = T0 + t0 * NTILE
                npair = (nb + 1) // 2
                wds = [min(2 * NTILE, (nb - 2 * p) * NTILE)
                       for p in range(npair)]
                prodss = [[None] * 5 for _ in range(npair)]

                def rhs_c(c, q):
                    if c < 3:
                        return gtab[:, q + c * W2: q + c * W2 + NTILE]
                    return gtac[:, q: q + NTILE]

                # m4 (lone j-tap, M=64) first: two concurrent col-tiled M=64
                # matmuls per (c, tile-pair) sharing ONE [128,512] PSUM region
                # (the pair's first tile on partitions 0-63, the second on
                # 64-127, same columns; weight cols 64-127 hold the duplicate)
                pst = [pspool.tile([128, 1024], F32, name=f"ps4_{p}",
                                   tag="ps") for p in range(npair)]
                for c in range(5):
                    k = 4 * 5 + c
                    wlo = wsb[:, k * 128: k * 128 + 64]
                    whi = wsb[:, k * 128 + 64: (k + 1) * 128]
                    tb = 0
                    while tb < nb:
                        q = T + tb * NTILE
                        pt = pst[tb // 2]
                        if tb + 1 < nb:
                            q1 = q + NTILE
                            if c < 4:
                                nc.tensor.matmul(
                                    pt[0:64, 0:NTILE], wlo, rhs_c(c, q),
                                    start=(c == 0), stop=False,
                                    tile_position=(0, 0),
                                )
                                nc.tensor.matmul(
                                    pt[64:128, 0:NTILE], whi, rhs_c(c, q1),
                                    start=(c == 0), stop=False,
                                    tile_position=(0, 64),
                                )
                            else:
                                nc.tensor.matmul(
                                    pt[0:64, 0:NTILE], wlo[0:64, :],
                                    gtab[0:64, q + 266: q + 266 + NTILE],
                                    start=False, stop=True,
                                    tile_position=(0, 0),
                                )
                                nc.tensor.matmul(
                                    pt[64:128, 0:NTILE], whi[64:128, :],
                                    gtab[64:128, q1 + 265: q1 + 265 + NTILE],
                                    start=False, stop=True,
                                    tile_position=(64, 64),
                                )
                            tb += 2
                        else:
                            # lone tile: old-style full-width, lower half used
                            if c < 4:
                                nc.tensor.matmul(
                                    pt[:, 0:NTILE],
                                    wsb[:, k * 128:(k + 1) * 128],
                                    rhs_c(c, q), start=(c == 0), stop=False,
                                )
                            else:
                                nc.tensor.matmul(
                                    pt[:, 0:NTILE],
                                    wsb[0:64, k * 128:(k + 1) * 128],
                                    gtab[0:64, q + 266: q + 266 + NTILE],
                                    start=False, stop=True,
                                )
                            tb += 1
                for p in range(npair):
                    Tp = T + p * 2 * NTILE
                    f = fpool.tile([128, 1024], F16, name=f"f4_{p}", tag="f")
                    pr = prodpool.tile([128, 1024], F16, name="pr4",
                                       tag="prod")
                    nc.scalar.activation(
                        f[:, 0:NTILE], pst[p][:, 0:NTILE], IDENT,
                        bias=bias_sb[:, 4:5],
                    )
                    nc.vector.tensor_tensor(
                        pr[0:64, 0:NTILE], f[0:64, 0:NTILE],
                        grab[0:64, Tp + 266: Tp + 266 + NTILE], op=MULT)
                    if wds[p] == 2 * NTILE:
                        nc.vector.tensor_tensor(
                            pr[64:128, 0:NTILE], f[64:128, 0:NTILE],
                            grab[64:128, Tp + NTILE + 265:
                                 Tp + NTILE + 265 + NTILE], op=MULT)
                    prodss[p][4] = pr

                for m in range(4):
                    pst = [pspool.tile([128, 1024], F32, name=f"ps{m}_{p}",
                                       tag="ps") for p in range(npair)]
                    for c in range(5):
                        k = m * 5 + c
                        lhsT = wsb[:, k * 128:(k + 1) * 128]
                        if c < 4:
                            for tb in range(nb):
                                out_ps = pst[tb // 2][:, (tb % 2) * NTILE:
                                                      (tb % 2 + 1) * NTILE]
                                nc.tensor.matmul(
                                    out_ps, lhsT, rhs_c(c, T + tb * NTILE),
                                    start=(c == 0), stop=False,
                                )
                        else:
                            # lone tap (2,2): two concurrent row-tiled K=64
                            # matmuls (partitions 0-63 / 64-127 hold the same
                            # weights; upper stack half is gt shifted +1, so
                            # column q+265 reads gt[q+266]) — two spatial
                            # tiles in one PE slot
                            tb = 0
                            while tb < nb:
                                q = T + tb * NTILE
                                out_ps = pst[tb // 2][:, (tb % 2) * NTILE:
                                                      (tb % 2 + 1) * NTILE]
                                if tb + 1 < nb:
                                    q1 = T + (tb + 1) * NTILE
                                    out_b = pst[(tb + 1) // 2][
                                        :, ((tb + 1) % 2) * NTILE:
                                        ((tb + 1) % 2 + 1) * NTILE]
                                    nc.tensor.matmul(
                                        out_ps, lhsT[0:64, :],
                                        gtab[0:64, q + 266: q + 266 + NTILE],
                                        start=False, stop=True,
                                        tile_position=(0, 0),
                                    )
                                    nc.tensor.matmul(
                                        out_b, lhsT[64:128, :],
                                        gtab[64:128, q1 + 265:
                                             q1 + 265 + NTILE],
                                        start=False, stop=True,
                                        tile_position=(64, 0),
                                    )
                                    tb += 2
                                else:
                                    nc.tensor.matmul(
                                        out_ps, lhsT[0:64, :],
                                        gtab[0:64, q + 266: q + 266 + NTILE],
                                        start=False, stop=True,
                                    )
                                    tb += 1
                    # psum + bias -> fp16 SBUF on the Scalar engine, then the
                    # gr-product immediately on DVE
                    for p in range(npair):
                        f = fpool.tile([128, 1024], F16, name=f"f{m}_{p}",
                                       tag="f")
                        nc.scalar.activation(
                            f[:, 0:wds[p]], pst[p][:, 0:wds[p]], IDENT,
                            bias=bias_sb[:, m:m + 1],
                        )
                        prodss[p][m] = emit_mult(
                            m, T + p * 2 * NTILE, wds[p], f)

                for p in range(npair):
                    emit_tree(T + p * 2 * NTILE, wds[p], prodss[p])
                t0 += nb
    nc.finalize()
    return nc


_NC = None


def _get_nc():
    global _NC
    if _NC is None:
        _NC = _build()
    return _NC


_RUN_KW = {}  # test harness can inject trace=True etc.
_LAST_RESULT = None


def kernel(gr, gt, Wc, bc):
    global _LAST_RESULT
    gr = np.ascontiguousarray(np.asarray(gr, dtype=np.float32))
    gt = np.ascontiguousarray(np.asarray(gt, dtype=np.float32))
    Wc = np.asarray(Wc, dtype=np.float32)
    bc = np.asarray(bc, dtype=np.float32)

    wb = _prep_w(Wc)
    bb = _prep_b(bc)
    in_maps = []
    for n in range(N):
        fgt = _flat_gt(gt[n])
        fgr = _flat_gr(gr[n])
        in_maps.append({
            "gtab_src": _stack(fgt, 0, 1, ml_dtypes.bfloat16),
            "gtac_src": _stack(fgt, 2, 134, ml_dtypes.bfloat16),
            "grab_src": _stack(fgr, 0, 1),
            "grac_src": _stack(fgr, 2, 134),
            "w_src": wb,
            "b_src": bb,
        })
    res = run_bass_kernel_spmd(
        _get_nc(), in_maps, core_ids=list(range(N)), **_RUN_KW
    )
    _LAST_RESULT = res

    # o_dst col t <-> flat pos 132 + t; output (h,w) at flat (h+1)*132+(w+1)
    hh = np.arange(H)
    cols = (hh * W2)[:, None] + (np.arange(W) + 1)[None, :]
    outs = []
    for n in range(N):
        O = res.results[n]["o_dst"].astype(np.float32)
        flat = O[:64] + O[64:]
        outs.append(flat[:, cols])
    return np.stack(outs).astype(np.float32)
